# revision 4
# baseline (speedup 1.0000x reference)
"""LSTM decoder kernel for Trainium2, 8 NeuronCores.

Strategy: data-parallel over batch (32 rows/core, no collectives).
Per-core matmuls are batch-major with 4x column tiling (each 32-wide
col-tile computes a different 256-col window of the gate dim, writing
PSUM partitions 32j:32j+32). The input projection x_proj (constant
across timesteps) is injected exactly in fp32 via identity-stationary
matmuls; the recurrent h @ W_hh.T runs in bf16. Cell state c stays
fp32. h is re-transposed each step with two PE transposes so it can be
the stationary operand of the next step's matmuls.

Layout (per core, batch b in [0,32), col-tile j in [0,4)):
  gate tile G in {i,f,o,g}: psum[32j+b, off_G+f] = pre_G[b, 256j+f]
  offsets: i->0, f->256, o->512, g->768  (bank0 = i,f ; bank1 = o,g)
  c/h tiles [128,256]: [32j+b, f] = state[b, 256j+f]
  hT chunks k: (T1 if k even else T2)[:, 32*(k//2)+ :32]
"""
import numpy as np
import ml_dtypes

import concourse.bass as bass
import concourse.mybir as mybir
import concourse.tile as tile
from concourse import bacc
from concourse import bass_utils

B, H, O, T, NCORES = 256, 1024, 512, 128, 8
BL = B // NCORES          # 32 batch rows per core
GOFF = {0: 0, 1: 256, 2: 768, 3: 512}   # gate G -> psum free offset (i,f,g,o)
BF16 = mybir.dt.bfloat16
F32 = mybir.dt.float32

_CACHE = {}


def _emit_gate_mms(nc, gates_ps, w_sb, t1, t2):
    """h @ W_hh.T contribution for one step (bf16, col-tiled)."""
    for k in range(8):
        tt = t1 if k % 2 == 0 else t2
        stat = tt[:, 32 * (k // 2):32 * (k // 2) + 32]
        for g in range(4):
            off = GOFF[g]
            for j in range(4):
                nc.tensor.matmul(
                    gates_ps[32 * j:32 * (j + 1), off:off + 256],
                    stat,
                    w_sb[k][:, 1024 * g + 256 * j:1024 * g + 256 * j + 256],
                    start=False,
                    stop=(k == 7 and j == 3 and g in (1, 3)),
                    tile_position=(0, 32 * j),
                    skip_group_check=True,
                )


def _emit_xp_mms(nc, gates_ps, eyef, xp_sb, final):
    """Exact fp32 x_proj injection; first MM per bank clears the bank."""
    for g in range(4):
        off = GOFF[g]
        for j in range(4):
            nc.tensor.matmul(
                gates_ps[32 * j:32 * (j + 1), off:off + 256],
                eyef[:, 32 * g:32 * g + 32],
                xp_sb[:, 256 * j:256 * j + 256],
                start=(g in (0, 2)),
                stop=(final and j == 3 and g in (1, 3)),
                tile_position=(0, 32 * j),
                skip_group_check=True,
            )


def _emit_y_mms(nc, y_ps, wl_sb, t1, t2):
    for k in range(8):
        tt = t1 if k % 2 == 0 else t2
        stat = tt[:, 32 * (k // 2):32 * (k // 2) + 32]
        for j in range(4):
            nc.tensor.matmul(
                y_ps[32 * j:32 * (j + 1), :],
                stat,
                wl_sb[:, 512 * k + 128 * j:512 * k + 128 * j + 128],
                start=(k == 0),
                stop=(k == 7 and j == 3),
                tile_position=(0, 32 * j),
                skip_group_check=True,
            )


def _build():
    nc = bacc.Bacc("TRN2", target_bir_lowering=False, debug=False,
                   num_devices=NCORES)
    w_d = nc.dram_tensor("W", [128, 8 * 4096], BF16, kind="ExternalInput").ap()
    wl_d = nc.dram_tensor("Wl", [128, 4096], BF16, kind="ExternalInput").ap()
    xp_d = nc.dram_tensor("xp", [128, 1024], F32, kind="ExternalInput").ap()
    eyeb_d = nc.dram_tensor("eyeb", [128, 128], BF16, kind="ExternalInput").ap()
    eyef_d = nc.dram_tensor("eyef", [128, 128], F32, kind="ExternalInput").ap()
    y_d = nc.dram_tensor("y", [T, 128, 128], F32, kind="ExternalOutput").ap()

    ACT = mybir.ActivationFunctionType

    with tile.TileContext(nc) as tc:
        with tc.tile_pool(name="stat", bufs=1) as statp, \
             tc.tile_pool(name="sb", bufs=2) as sb, \
             tc.tile_pool(name="ps", bufs=2, space="PSUM") as ps:
            w_sb = []
            for k in range(8):
                wk = statp.tile([128, 4096], BF16, tag=f"W{k}")
                nc.sync.dma_start(wk[:], w_d[:, 4096 * k:4096 * (k + 1)])
                w_sb.append(wk)
            wl_sb = statp.tile([128, 4096], BF16, tag="Wl")
            nc.sync.dma_start(wl_sb[:], wl_d)
            xp_sb = statp.tile([128, 1024], F32, tag="xp")
            nc.sync.dma_start(xp_sb[:], xp_d)
            eyeb = statp.tile([128, 128], BF16, tag="eyeb")
            nc.sync.dma_start(eyeb[:], eyeb_d)
            eyef = statp.tile([128, 128], F32, tag="eyef")
            nc.sync.dma_start(eyef[:], eyef_d)
            c_sb = statp.tile([128, 256], F32, tag="c")
            nc.gpsimd.memset(c_sb[:], 0.0)

            mult = mybir.AluOpType.mult
            addop = mybir.AluOpType.add

            t1_prev = t2_prev = None
            gates_cur = ps.tile([128, 1024], F32, tag="gates")
            _emit_xp_mms(nc, gates_cur, eyef, xp_sb, final=True)

            for t in range(T):
                if t > 0:
                    _emit_gate_mms(nc, gates_cur, w_sb, t1_prev, t2_prev)

                sig = sb.tile([128, 768], F32, tag="sig")
                nc.scalar.activation(sig[:], gates_cur[:, 0:768], ACT.Sigmoid)
                gt = sb.tile([128, 256], F32, tag="gt")
                nc.scalar.activation(gt[:], gates_cur[:, 768:1024], ACT.Tanh)

                # y for the previous step + x_proj for the next step fill the
                # PE idle window while ACT/DVE run this step's tail.
                if t > 0:
                    y_ps = ps.tile([128, 128], F32, tag="y")
                    _emit_y_mms(nc, y_ps, wl_sb, t1_prev, t2_prev)
                    y_sb = sb.tile([128, 128], F32, tag="ysb")
                    nc.vector.tensor_copy(y_sb[:], y_ps[:])
                    nc.sync.dma_start(y_d[t - 1], y_sb[:])
                if t < T - 1:
                    gates_next = ps.tile([128, 1024], F32, tag="gates")
                    _emit_xp_mms(nc, gates_next, eyef, xp_sb, final=False)

                tmp = sb.tile([128, 256], F32, tag="tmp")
                nc.vector.tensor_tensor(tmp[:], sig[:, 0:256], gt[:], mult)
                nc.vector.tensor_tensor(c_sb[:], sig[:, 256:512], c_sb[:], mult)
                nc.vector.tensor_tensor(c_sb[:], c_sb[:], tmp[:], addop)
                th = sb.tile([128, 256], F32, tag="th")
                nc.scalar.activation(th[:], c_sb[:], ACT.Tanh)
                h_sb = sb.tile([128, 256], BF16, tag="h")
                nc.vector.tensor_tensor(h_sb[:], sig[:, 512:768], th[:], mult)

                tp = ps.tile([128, 256], BF16, tag="tp")
                nc.tensor.transpose(tp[:, 0:128], h_sb[:, 0:128], eyeb[:])
                nc.tensor.transpose(tp[:, 128:256], h_sb[:, 128:256], eyeb[:])
                t1 = sb.tile([128, 128], BF16, tag="t1")
                nc.vector.tensor_copy(t1[:], tp[:, 0:128])
                t2 = sb.tile([128, 128], BF16, tag="t2")
                nc.vector.tensor_copy(t2[:], tp[:, 128:256])

                t1_prev, t2_prev = t1, t2
                if t < T - 1:
                    gates_cur = gates_next

            y_ps = ps.tile([128, 128], F32, tag="y")
            _emit_y_mms(nc, y_ps, wl_sb, t1_prev, t2_prev)
            y_sb = sb.tile([128, 128], F32, tag="ysb")
            nc.vector.tensor_copy(y_sb[:], y_ps[:])
            nc.sync.dma_start(y_d[T - 1], y_sb[:])

    nc.compile()
    return nc


def _prep_inputs(C, W_ih, W_hh, b_ih, b_hh, W_lin):
    xp = np.asarray(C, np.float32) @ np.asarray(W_ih, np.float32).T
    xp = xp + np.asarray(b_ih, np.float32) + np.asarray(b_hh, np.float32)
    w_dev = np.ascontiguousarray(
        np.asarray(W_hh, np.float32).T.reshape(8, 128, 4096)
        .transpose(1, 0, 2).reshape(128, 8 * 4096)).astype(ml_dtypes.bfloat16)
    wl_dev = np.ascontiguousarray(
        np.asarray(W_lin, np.float32).T.reshape(8, 128, 512)
        .transpose(1, 0, 2).reshape(128, 4096)).astype(ml_dtypes.bfloat16)
    eyeb = np.eye(128, dtype=ml_dtypes.bfloat16)
    eyef = np.eye(128, dtype=np.float32)
    in_maps = []
    for c in range(NCORES):
        xp_c = np.ascontiguousarray(
            xp[BL * c:BL * (c + 1)].reshape(BL, 4, 1024)
            .transpose(1, 0, 2).reshape(128, 1024))
        in_maps.append({"W": w_dev, "Wl": wl_dev, "xp": xp_c,
                        "eyeb": eyeb, "eyef": eyef})
    return in_maps


def kernel(C, W_ih, W_hh, b_ih, b_hh, W_lin, b_lin, max_seq_len):
    assert int(max_seq_len) == T and C.shape == (B, H)
    if "nc" not in _CACHE:
        _CACHE["nc"] = _build()
    nc = _CACHE["nc"]
    in_maps = _prep_inputs(C, W_ih, W_hh, b_ih, b_hh, W_lin)
    res = bass_utils.run_bass_kernel_spmd(
        nc, in_maps, core_ids=list(range(NCORES)))
    out = np.empty((T, B, O), np.float32)
    blin = np.asarray(b_lin, np.float32)
    for c in range(NCORES):
        yc = res.results[c]["y"]          # [T, 128, 128]
        out[:, BL * c:BL * (c + 1), :] = (
            yc.reshape(T, 4, BL, 128).transpose(0, 2, 1, 3).reshape(T, BL, O)
            + blin)
    return out


# revision 6
# speedup vs baseline: 1.1934x; 1.1934x over previous
"""LSTM decoder kernel for Trainium2, 8 NeuronCores.

Strategy: data-parallel over batch (32 rows/core, no collectives).
Per-core matmuls are batch-major with 4x column tiling (each 32-wide
col-tile computes a different 256-col window of the gate dim, writing
PSUM partitions 32j:32j+32). The input projection x_proj (constant
across timesteps) is injected exactly in fp32 via identity-stationary
matmuls; the recurrent h @ W_hh.T runs in bf16. Cell state c stays
fp32. h is re-transposed each step with two PE transposes so it can be
the stationary operand of the next step's matmuls.

Layout (per core, batch b in [0,32), col-tile j in [0,4)):
  gate tile G in {i,f,o,g}: psum[32j+b, off_G+f] = pre_G[b, 256j+f]
  offsets: i->0, f->256, o->512, g->768  (bank0 = i,f ; bank1 = o,g)
  c/h tiles [128,256]: [32j+b, f] = state[b, 256j+f]
  hT chunks k: (T1 if k even else T2)[:, 32*(k//2)+ :32]
"""
import numpy as np
import ml_dtypes

import concourse.bass as bass
import concourse.mybir as mybir
import concourse.tile as tile
from concourse import bacc
from concourse import bass_utils

B, H, O, T, NCORES = 256, 1024, 512, 128, 8
BL = B // NCORES          # 32 batch rows per core
GOFF = {0: 0, 1: 256, 2: 768, 3: 512}   # gate G -> psum free offset (i,f,g,o)
BF16 = mybir.dt.bfloat16
F32 = mybir.dt.float32

_CACHE = {}


def _emit_gate_mms(nc, gates_ps, w_sb, t1, t2):
    """h @ W_hh.T contribution for one step (bf16, col-tiled).

    W columns are host-reordered so each (bank, col-tile) pair is one
    contiguous 512-wide window covering two gates."""
    for k in range(8):
        tt = t1 if k % 2 == 0 else t2
        stat = tt[:, 32 * (k // 2):32 * (k // 2) + 32]
        for bank in range(2):
            for j in range(4):
                nc.tensor.matmul(
                    gates_ps[32 * j:32 * (j + 1), 512 * bank:512 * (bank + 1)],
                    stat,
                    w_sb[k][:, 2048 * bank + 512 * j:2048 * bank + 512 * (j + 1)],
                    start=False,
                    stop=(k == 7 and j == 3),
                    tile_position=(0, 32 * j),
                    skip_group_check=True,
                )


def _emit_xp_mms(nc, gates_ps, eyef, xp_sb, final):
    """Exact fp32 x_proj injection; each MM clears its (bank, col-tile)."""
    for bank in range(2):
        for j in range(4):
            nc.tensor.matmul(
                gates_ps[32 * j:32 * (j + 1), 512 * bank:512 * (bank + 1)],
                eyef[:, 32 * bank:32 * bank + 32],
                xp_sb[:, (bank * 4 + j) * 512:(bank * 4 + j + 1) * 512],
                start=True,
                stop=(final and j == 3),
                tile_position=(0, 32 * j),
                skip_group_check=True,
            )


def _emit_y_mms(nc, y_ps, wl_sb, t1, t2):
    for k in range(8):
        tt = t1 if k % 2 == 0 else t2
        stat = tt[:, 32 * (k // 2):32 * (k // 2) + 32]
        for j in range(4):
            nc.tensor.matmul(
                y_ps[32 * j:32 * (j + 1), :],
                stat,
                wl_sb[:, 512 * k + 128 * j:512 * k + 128 * j + 128],
                start=(k == 0),
                stop=(k == 7 and j == 3),
                tile_position=(0, 32 * j),
                skip_group_check=True,
            )


def _build(steps=T):
    nc = bacc.Bacc("TRN2", target_bir_lowering=False, debug=False,
                   num_devices=NCORES)
    w_d = nc.dram_tensor("W", [128, 8 * 4096], BF16, kind="ExternalInput").ap()
    wl_d = nc.dram_tensor("Wl", [128, 4096], BF16, kind="ExternalInput").ap()
    xp_d = nc.dram_tensor("xp", [128, 4096], F32, kind="ExternalInput").ap()
    eyeb_d = nc.dram_tensor("eyeb", [128, 128], BF16, kind="ExternalInput").ap()
    eyef_d = nc.dram_tensor("eyef", [128, 128], F32, kind="ExternalInput").ap()
    y_d = nc.dram_tensor("y", [T, 128, 128], F32, kind="ExternalOutput").ap()

    ACT = mybir.ActivationFunctionType

    with tile.TileContext(nc) as tc:
        with tc.tile_pool(name="stat", bufs=1) as statp, \
             tc.tile_pool(name="sb", bufs=2) as sb, \
             tc.tile_pool(name="ps", bufs=2, space="PSUM") as ps:
            w_sb = []
            for k in range(8):
                wk = statp.tile([128, 4096], BF16, tag=f"W{k}")
                nc.sync.dma_start(wk[:], w_d[:, 4096 * k:4096 * (k + 1)])
                w_sb.append(wk)
            wl_sb = statp.tile([128, 4096], BF16, tag="Wl")
            nc.sync.dma_start(wl_sb[:], wl_d)
            xp_sb = statp.tile([128, 4096], F32, tag="xp")
            nc.sync.dma_start(xp_sb[:], xp_d)
            eyeb = statp.tile([128, 128], BF16, tag="eyeb")
            nc.sync.dma_start(eyeb[:], eyeb_d)
            eyef = statp.tile([128, 128], F32, tag="eyef")
            nc.sync.dma_start(eyef[:], eyef_d)
            c_sb = statp.tile([128, 256], F32, tag="c")
            nc.gpsimd.memset(c_sb[:], 0.0)

            mult = mybir.AluOpType.mult
            addop = mybir.AluOpType.add

            t1_prev = t2_prev = None
            gates_cur = ps.tile([128, 1024], F32, tag="gates")
            _emit_xp_mms(nc, gates_cur, eyef, xp_sb, final=True)

            for t in range(steps):
                if t > 0:
                    _emit_gate_mms(nc, gates_cur, w_sb, t1_prev, t2_prev)

                sig = sb.tile([128, 768], F32, tag="sig")
                nc.scalar.activation(sig[:], gates_cur[:, 0:768], ACT.Sigmoid)
                gt = sb.tile([128, 256], F32, tag="gt")
                nc.scalar.activation(gt[:], gates_cur[:, 768:1024], ACT.Tanh)

                # y for the previous step + x_proj for the next step fill the
                # PE idle window while ACT/DVE run this step's tail.
                if t > 0:
                    y_ps = ps.tile([128, 128], F32, tag="y")
                    _emit_y_mms(nc, y_ps, wl_sb, t1_prev, t2_prev)
                    y_sb = sb.tile([128, 128], F32, tag="ysb")
                    nc.vector.tensor_copy(y_sb[:], y_ps[:])
                    nc.sync.dma_start(y_d[t - 1], y_sb[:])
                if t < steps - 1:
                    gates_next = ps.tile([128, 1024], F32, tag="gates")
                    _emit_xp_mms(nc, gates_next, eyef, xp_sb, final=False)

                tmp = sb.tile([128, 256], F32, tag="tmp")
                nc.vector.tensor_tensor(tmp[:], sig[:, 0:256], gt[:], mult)
                nc.vector.tensor_tensor(c_sb[:], sig[:, 256:512], c_sb[:], mult)
                nc.vector.tensor_tensor(c_sb[:], c_sb[:], tmp[:], addop)
                th = sb.tile([128, 256], F32, tag="th")
                nc.scalar.activation(th[:], c_sb[:], ACT.Tanh)
                h_sb = sb.tile([128, 256], BF16, tag="h")
                nc.vector.tensor_tensor(h_sb[:], sig[:, 512:768], th[:], mult)

                tp = ps.tile([128, 256], BF16, tag="tp")
                nc.tensor.transpose(tp[:, 0:128], h_sb[:, 0:128], eyeb[:])
                nc.tensor.transpose(tp[:, 128:256], h_sb[:, 128:256], eyeb[:])
                t1 = sb.tile([128, 128], BF16, tag="t1")
                nc.vector.tensor_copy(t1[:], tp[:, 0:128])
                t2 = sb.tile([128, 128], BF16, tag="t2")
                nc.vector.tensor_copy(t2[:], tp[:, 128:256])

                t1_prev, t2_prev = t1, t2
                if t < steps - 1:
                    gates_cur = gates_next

            y_ps = ps.tile([128, 128], F32, tag="y")
            _emit_y_mms(nc, y_ps, wl_sb, t1_prev, t2_prev)
            y_sb = sb.tile([128, 128], F32, tag="ysb")
            nc.vector.tensor_copy(y_sb[:], y_ps[:])
            nc.sync.dma_start(y_d[steps - 1], y_sb[:])

    nc.compile()
    return nc


def _colmap():
    """Map device gate-column w -> original gate column (psum layout i,f|o,g)."""
    m = np.empty(4096, np.int64)
    ar = np.arange(256)
    for j in range(4):
        m[512 * j:512 * j + 256] = 0 * 1024 + 256 * j + ar          # i
        m[512 * j + 256:512 * (j + 1)] = 1 * 1024 + 256 * j + ar    # f
        m[2048 + 512 * j:2048 + 512 * j + 256] = 3 * 1024 + 256 * j + ar   # o
        m[2048 + 512 * j + 256:2048 + 512 * (j + 1)] = 2 * 1024 + 256 * j + ar  # g
    return m


def _prep_inputs(C, W_ih, W_hh, b_ih, b_hh, W_lin):
    xp = np.asarray(C, np.float32) @ np.asarray(W_ih, np.float32).T
    xp = xp + np.asarray(b_ih, np.float32) + np.asarray(b_hh, np.float32)
    cm = _colmap()
    w_perm = np.asarray(W_hh, np.float32).T[:, cm]
    w_dev = np.ascontiguousarray(
        w_perm.reshape(8, 128, 4096)
        .transpose(1, 0, 2).reshape(128, 8 * 4096)).astype(ml_dtypes.bfloat16)
    wl_dev = np.ascontiguousarray(
        np.asarray(W_lin, np.float32).T.reshape(8, 128, 512)
        .transpose(1, 0, 2).reshape(128, 4096)).astype(ml_dtypes.bfloat16)
    eyeb = np.eye(128, dtype=ml_dtypes.bfloat16)
    eyef = np.eye(128, dtype=np.float32)
    in_maps = []
    for c in range(NCORES):
        xpb = xp[BL * c:BL * (c + 1)][:, cm]   # [32, 4096] in device col order
        xp_c = np.zeros((128, 4096), np.float32)
        for bank in range(2):
            xp_c[32 * bank:32 * (bank + 1), 2048 * bank:2048 * (bank + 1)] = \
                xpb[:, 2048 * bank:2048 * (bank + 1)]
        in_maps.append({"W": w_dev, "Wl": wl_dev, "xp": xp_c,
                        "eyeb": eyeb, "eyef": eyef})
    return in_maps


def kernel(C, W_ih, W_hh, b_ih, b_hh, W_lin, b_lin, max_seq_len):
    assert int(max_seq_len) == T and C.shape == (B, H)
    if "nc" not in _CACHE:
        _CACHE["nc"] = _build()
    nc = _CACHE["nc"]
    in_maps = _prep_inputs(C, W_ih, W_hh, b_ih, b_hh, W_lin)
    res = bass_utils.run_bass_kernel_spmd(
        nc, in_maps, core_ids=list(range(NCORES)))
    out = np.empty((T, B, O), np.float32)
    blin = np.asarray(b_lin, np.float32)
    for c in range(NCORES):
        yc = res.results[c]["y"]          # [T, 128, 128]
        out[:, BL * c:BL * (c + 1), :] = (
            yc.reshape(T, 4, BL, 128).transpose(0, 2, 1, 3).reshape(T, BL, O)
            + blin)
    return out


# revision 7
# speedup vs baseline: 2259.7562x; 1893.4768x over previous
"""LSTM decoder kernel for Trainium2, 8 NeuronCores.

Strategy: data-parallel over batch (32 rows/core, no collectives).
Per-core matmuls are batch-major with 4x column tiling (each 32-wide
col-tile computes a different 256-col window of the gate dim, writing
PSUM partitions 32j:32j+32). The input projection x_proj (constant
across timesteps) is injected exactly in fp32 via identity-stationary
matmuls; the recurrent h @ W_hh.T runs in bf16. Cell state c stays
fp32. h is re-transposed each step with two PE transposes so it can be
the stationary operand of the next step's matmuls.

Layout (per core, batch b in [0,32), col-tile j in [0,4)):
  gate tile G in {i,f,o,g}: psum[32j+b, off_G+f] = pre_G[b, 256j+f]
  offsets: i->0, f->256, o->512, g->768  (bank0 = i,f ; bank1 = o,g)
  c/h tiles [128,256]: [32j+b, f] = state[b, 256j+f]
  hT chunks k: (T1 if k even else T2)[:, 32*(k//2)+ :32]
"""
import numpy as np
import ml_dtypes

import concourse.bass as bass
import concourse.mybir as mybir
import concourse.tile as tile
from concourse import bacc
from concourse import bass_utils

B, H, O, T, NCORES = 256, 1024, 512, 128, 8
BL = B // NCORES          # 32 batch rows per core
GOFF = {0: 0, 1: 256, 2: 768, 3: 512}   # gate G -> psum free offset (i,f,g,o)
BF16 = mybir.dt.bfloat16
F32 = mybir.dt.float32

_CACHE = {}


def _emit_gate_mms(nc, gates_ps, w_sb, t1, t2):
    """h @ W_hh.T contribution for one step (bf16, col-tiled).

    W columns are host-reordered so each (bank, col-tile) pair is one
    contiguous 512-wide window covering two gates."""
    for k in range(8):
        tt = t1 if k % 2 == 0 else t2
        stat = tt[:, 32 * (k // 2):32 * (k // 2) + 32]
        for bank in range(2):
            for j in range(4):
                nc.tensor.matmul(
                    gates_ps[32 * j:32 * (j + 1), 512 * bank:512 * (bank + 1)],
                    stat,
                    w_sb[k][:, 2048 * bank + 512 * j:2048 * bank + 512 * (j + 1)],
                    start=False,
                    stop=(k == 7 and j == 3),
                    tile_position=(0, 32 * j),
                    skip_group_check=True,
                )


def _emit_xp_mms(nc, gates_ps, eyef, xp_sb, final):
    """Exact fp32 x_proj injection; each MM clears its (bank, col-tile)."""
    for bank in range(2):
        for j in range(4):
            nc.tensor.matmul(
                gates_ps[32 * j:32 * (j + 1), 512 * bank:512 * (bank + 1)],
                eyef[:, 32 * bank:32 * bank + 32],
                xp_sb[:, (bank * 4 + j) * 512:(bank * 4 + j + 1) * 512],
                start=True,
                stop=(final and j == 3),
                tile_position=(0, 32 * j),
                skip_group_check=True,
            )


def _emit_y_mms(nc, y_ps, wl_sb, t1, t2):
    for k in range(8):
        tt = t1 if k % 2 == 0 else t2
        stat = tt[:, 32 * (k // 2):32 * (k // 2) + 32]
        for j in range(4):
            nc.tensor.matmul(
                y_ps[32 * j:32 * (j + 1), :],
                stat,
                wl_sb[:, 512 * k + 128 * j:512 * k + 128 * j + 128],
                start=(k == 0),
                stop=(k == 7 and j == 3),
                tile_position=(0, 32 * j),
                skip_group_check=True,
            )


def _build(steps=T):
    nc = bacc.Bacc("TRN2", target_bir_lowering=False, debug=False,
                   num_devices=NCORES)
    w_d = nc.dram_tensor("W", [128, 8 * 4096], BF16, kind="ExternalInput").ap()
    wl_d = nc.dram_tensor("Wl", [128, 4096], BF16, kind="ExternalInput").ap()
    xp_d = nc.dram_tensor("xp", [128, 4096], F32, kind="ExternalInput").ap()
    eyeb_d = nc.dram_tensor("eyeb", [128, 128], BF16, kind="ExternalInput").ap()
    eyef_d = nc.dram_tensor("eyef", [128, 128], F32, kind="ExternalInput").ap()
    y_d = nc.dram_tensor("y", [T, 128, 128], F32, kind="ExternalOutput").ap()

    ACT = mybir.ActivationFunctionType

    with tile.TileContext(nc) as tc:
        with tc.tile_pool(name="stat", bufs=1) as statp, \
             tc.tile_pool(name="sb", bufs=2) as sb, \
             tc.tile_pool(name="ps", bufs=2, space="PSUM") as ps:
            w_sb = []
            for k in range(8):
                wk = statp.tile([128, 4096], BF16, tag=f"W{k}")
                nc.sync.dma_start(wk[:], w_d[:, 4096 * k:4096 * (k + 1)])
                w_sb.append(wk)
            wl_sb = statp.tile([128, 4096], BF16, tag="Wl")
            nc.sync.dma_start(wl_sb[:], wl_d)
            xp_sb = statp.tile([128, 4096], F32, tag="xp")
            nc.sync.dma_start(xp_sb[:], xp_d)
            eyeb = statp.tile([128, 128], BF16, tag="eyeb")
            nc.sync.dma_start(eyeb[:], eyeb_d)
            eyef = statp.tile([128, 128], F32, tag="eyef")
            nc.sync.dma_start(eyef[:], eyef_d)
            c_sb = statp.tile([128, 256], F32, tag="c")
            nc.gpsimd.memset(c_sb[:], 0.0)

            mult = mybir.AluOpType.mult
            addop = mybir.AluOpType.add

            t1_prev = t2_prev = None
            gates_cur = ps.tile([128, 1024], F32, tag="gates")
            _emit_xp_mms(nc, gates_cur, eyef, xp_sb, final=True)

            for t in range(steps):
                if t > 0:
                    _emit_gate_mms(nc, gates_cur, w_sb, t1_prev, t2_prev)

                sig = sb.tile([128, 768], F32, tag="sig")
                nc.scalar.activation(sig[:], gates_cur[:, 0:768], ACT.Sigmoid)
                gt = sb.tile([128, 256], F32, tag="gt")
                nc.scalar.activation(gt[:], gates_cur[:, 768:1024], ACT.Tanh)

                # y for the previous step + x_proj for the next step fill the
                # PE idle window while ACT/DVE run this step's tail.
                if t > 0:
                    y_ps = ps.tile([128, 128], F32, tag="y")
                    _emit_y_mms(nc, y_ps, wl_sb, t1_prev, t2_prev)
                    y_sb = sb.tile([128, 128], F32, tag="ysb")
                    nc.vector.tensor_copy(y_sb[:], y_ps[:])
                    nc.sync.dma_start(y_d[t - 1], y_sb[:])
                if t < steps - 1:
                    gates_next = ps.tile([128, 1024], F32, tag="gates")
                    _emit_xp_mms(nc, gates_next, eyef, xp_sb, final=False)

                tmp = sb.tile([128, 256], F32, tag="tmp")
                nc.vector.tensor_tensor(tmp[:], sig[:, 0:256], gt[:], mult)
                nc.vector.tensor_tensor(c_sb[:], sig[:, 256:512], c_sb[:], mult)
                nc.vector.tensor_tensor(c_sb[:], c_sb[:], tmp[:], addop)
                th = sb.tile([128, 256], F32, tag="th")
                nc.scalar.activation(th[:], c_sb[:], ACT.Tanh)
                h_sb = sb.tile([128, 256], BF16, tag="h")
                nc.vector.tensor_tensor(h_sb[:], sig[:, 512:768], th[:], mult)

                tp = ps.tile([128, 256], BF16, tag="tp")
                nc.tensor.transpose(tp[:, 0:128], h_sb[:, 0:128], eyeb[:])
                nc.tensor.transpose(tp[:, 128:256], h_sb[:, 128:256], eyeb[:])
                t1 = sb.tile([128, 128], BF16, tag="t1")
                nc.vector.tensor_copy(t1[:], tp[:, 0:128])
                t2 = sb.tile([128, 128], BF16, tag="t2")
                nc.vector.tensor_copy(t2[:], tp[:, 128:256])

                t1_prev, t2_prev = t1, t2
                if t < steps - 1:
                    gates_cur = gates_next

            y_ps = ps.tile([128, 128], F32, tag="y")
            _emit_y_mms(nc, y_ps, wl_sb, t1_prev, t2_prev)
            y_sb = sb.tile([128, 128], F32, tag="ysb")
            nc.vector.tensor_copy(y_sb[:], y_ps[:])
            nc.sync.dma_start(y_d[steps - 1], y_sb[:])

    nc.compile()
    return nc


def _colmap():
    """Map device gate-column w -> original gate column (psum layout i,f|o,g)."""
    m = np.empty(4096, np.int64)
    ar = np.arange(256)
    for j in range(4):
        m[512 * j:512 * j + 256] = 0 * 1024 + 256 * j + ar          # i
        m[512 * j + 256:512 * (j + 1)] = 1 * 1024 + 256 * j + ar    # f
        m[2048 + 512 * j:2048 + 512 * j + 256] = 3 * 1024 + 256 * j + ar   # o
        m[2048 + 512 * j + 256:2048 + 512 * (j + 1)] = 2 * 1024 + 256 * j + ar  # g
    return m


def _prep_inputs(C, W_ih, W_hh, b_ih, b_hh, W_lin):
    xp = np.asarray(C, np.float32) @ np.asarray(W_ih, np.float32).T
    xp = xp + np.asarray(b_ih, np.float32) + np.asarray(b_hh, np.float32)
    cm = _colmap()
    w_perm = np.asarray(W_hh, np.float32).T[:, cm]
    w_dev = np.ascontiguousarray(
        w_perm.reshape(8, 128, 4096)
        .transpose(1, 0, 2).reshape(128, 8 * 4096)).astype(ml_dtypes.bfloat16)
    wl_dev = np.ascontiguousarray(
        np.asarray(W_lin, np.float32).T.reshape(8, 128, 512)
        .transpose(1, 0, 2).reshape(128, 4096)).astype(ml_dtypes.bfloat16)
    eyeb = np.eye(128, dtype=ml_dtypes.bfloat16)
    eyef = np.eye(128, dtype=np.float32)
    in_maps = []
    for c in range(NCORES):
        xpb = xp[BL * c:BL * (c + 1)][:, cm]   # [32, 4096] in device col order
        xp_c = np.zeros((128, 4096), np.float32)
        for bank in range(2):
            xp_c[32 * bank:32 * (bank + 1), 2048 * bank:2048 * (bank + 1)] = \
                xpb[:, 2048 * bank:2048 * (bank + 1)]
        in_maps.append({"W": w_dev, "Wl": wl_dev, "xp": xp_c,
                        "eyeb": eyeb, "eyef": eyef})
    return in_maps


def kernel(C, W_ih, W_hh, b_ih, b_hh, W_lin, b_lin, max_seq_len):
    assert int(max_seq_len) == T and C.shape == (B, H)
    if "nc" not in _CACHE:
        _CACHE["nc"] = _build()
    nc = _CACHE["nc"]
    in_maps = _prep_inputs(C, W_ih, W_hh, b_ih, b_hh, W_lin)
    try:
        res = bass_utils.run_bass_kernel_spmd(
            nc, in_maps, core_ids=list(range(NCORES)))
    except Exception:
        # transient NRT faults have been observed on this fabric; retry once
        res = bass_utils.run_bass_kernel_spmd(
            nc, in_maps, core_ids=list(range(NCORES)))
    out = np.empty((T, B, O), np.float32)
    blin = np.asarray(b_lin, np.float32)
    for c in range(NCORES):
        yc = res.results[c]["y"]          # [T, 128, 128]
        out[:, BL * c:BL * (c + 1), :] = (
            yc.reshape(T, 4, BL, 128).transpose(0, 2, 1, 3).reshape(T, BL, O)
            + blin)
    return out
